# revision 1
# baseline (speedup 1.0000x reference)
"""GAT (2-layer: 2-head concat then 1-head) + global mean pool + MLP on 8
Trainium2 cores.

Sharding: nodes and their incoming edges are partitioned across 8 cores by
destination (6250 own nodes/core, padded to 6272 = 49 chunks of 128).  Nodes
are re-ordered per core by descending in-degree so fixed-size neighbor-rank
tiles stay tight.  Layer-1's gather table (h1 = x@W1aug, bf16, with the
attention score columns folded in as extra output columns of the augmented
weight matrix) is computed replicated on every core - cheaper than
all-gathering a 32MB table.  Layer-2's table is built from each core's own
layer-1 output and AllGathered (13MB bf16).

Edge aggregation: dma_gather pulls neighbor rows into a node-per-partition /
neighbor-rank-per-free-axis layout (two gathers per group: the 50176-row table
is split in two halves because gather indices are int16).  Pad slots carry
idx=-1 (skipped by the DGE; stale data is masked out).  Attention:
e = leaky_relu(asrc[src]+adst[dst]) via one ACT Prelu with per-partition bias,
exp on ACT, mask+denominator on DVE; softmax normalization is folded into a
single per-node reciprocal scale after the weighted sum (exact - no max
subtraction needed, |e| <= ~15 in fp32).  Weighted sums: per-rank ACT scaled
copies into a transposed-packed f32 tile, then one contiguous DVE reduction.
Mean-pool via one-hot PE matmuls + AllReduce; the small MLP runs on-device.
"""
import os
import sys
from contextlib import ExitStack

import numpy as np

NC = 8
N = 50000
E = 800000
IN_CH = 128
HID = 128
G = 1024
NPC = N // NC          # 6250
KCH = 49
NPCP = KCH * 128       # 6272
TROWS = NC * NPCP      # 50176
HALF = TROWS // 2      # 25088
ELEM1 = 384            # bf16: [h(256) | fsrc1 fsrc2 fdst1 fdst2 | pad]
ELEM2 = 256            # bf16: [h2(128) | fsrc2 fdst2 | pad]
NEG_SLOPE = 0.2
EPS = 1e-30
R_MAX = int(os.environ.get('GAT_RMAX', '24'))  # rank capacity per super-gather

_VERBOSE = bool(int(os.environ.get("GAT_VERBOSE", "0")))
LAST_EXEC_TIME_NS = None


def _log(*a):
    if _VERBOSE:
        print("[kernel]", *a, flush=True)


# --------------------------------------------------------------------------
# Host-side preprocessing
# --------------------------------------------------------------------------

def _prep(x, edge_index, batch, W1, att_src1, att_dst1, W2, att_src2, att_dst2):
    src = np.concatenate([edge_index[0], np.arange(N, dtype=np.int64)])
    dst = np.concatenate([edge_index[1], np.arange(N, dtype=np.int64)])
    deg = np.bincount(dst, minlength=N)

    core_of = np.arange(N) // NPC
    # a source's table half is determined by its core (cores 0-3 -> low), so
    # per-half in-degrees are known before permuting; grouping nodes by the
    # max of the two halves' counts minimizes padded neighbor-rank capacity
    halfv_pre = (core_of[src] >= NC // 2).astype(np.int64)
    cnt_pre = np.zeros((N, 2), dtype=np.int64)
    np.add.at(cnt_pre, (dst, halfv_pre), 1)
    sort_key = np.maximum(cnt_pre[:, 0], cnt_pre[:, 1])
    pos = np.empty(N, dtype=np.int64)
    for c in range(NC):
        own = slice(c * NPC, (c + 1) * NPC)
        order = np.argsort(-sort_key[own], kind="stable")
        pos[c * NPC + order] = np.arange(NPC)
    rowid = core_of * NPCP + pos

    srow = rowid[src]
    halfv = (srow >= HALF).astype(np.int64)

    keys = dst * 2 + halfv
    o2 = np.argsort(keys, kind="stable")
    ks = keys[o2]
    grp_first = np.r_[True, np.diff(ks) != 0]
    grp_start_idx = np.flatnonzero(grp_first)
    grp_len = np.diff(np.r_[grp_start_idx, len(ks)])
    rank = np.arange(len(ks)) - np.repeat(grp_start_idx, grp_len)

    e_dst = dst[o2]
    e_half = halfv[o2]
    e_val = (srow[o2] - e_half * HALF).astype(np.int16)
    e_core = core_of[e_dst]
    e_pos = pos[e_dst]
    e_k = e_pos // 128
    e_p = e_pos % 128

    cnt = np.zeros((N, 2), dtype=np.int64)
    np.add.at(cnt, (dst, halfv), 1)
    D_uni = np.zeros((KCH, 2), dtype=np.int64)
    np.maximum.at(D_uni, (pos // 128, 0), cnt[:, 0])
    np.maximum.at(D_uni, (pos // 128, 1), cnt[:, 1])

    blk_off = np.zeros((KCH, 2), dtype=np.int64)
    blk_off[1:, 0] = np.cumsum(D_uni[:-1, 0]) * 128
    blk_off[1:, 1] = np.cumsum(D_uni[:-1, 1]) * 128
    LEN = [int(D_uni[:, h].sum()) * 128 for h in (0, 1)]
    col_off = np.zeros((KCH, 2), dtype=np.int64)
    flat = D_uni.reshape(-1)
    col_off.reshape(-1)[1:] = np.cumsum(flat)[:-1]
    CTOT = int(flat.sum())

    supers = {0: [], 1: []}
    for h in (0, 1):
        cur, cur_r = [], 0
        for k in range(KCH):
            d = int(D_uni[k, h])
            if d == 0:
                continue
            if cur and cur_r + d > R_MAX:
                supers[h].append(cur)
                cur, cur_r = [], 0
            cur.append(k)
            cur_r += d
        if cur:
            supers[h].append(cur)

    # per-(k,h) valid counts, equalized across cores
    nv = np.zeros((NC, KCH, 2), dtype=np.int64)
    np.add.at(nv, (e_core, e_k, e_half), 1)
    K_valid = nv.max(axis=0)                 # [KCH, 2]

    idx_streams = []
    masks = []
    own_idx = []
    is_lo = []
    batchp = []
    invcnt = []
    gcnt = np.bincount(batch, minlength=G).astype(np.float32)
    gcnt_c = np.maximum(gcnt, 1.0)

    def _wrap16(lin):
        assert len(lin) % 16 == 0
        return np.tile(lin.reshape(-1, 16).T, (8, 1)).copy()

    for c in range(NC):
        sel = e_core == c
        streams = []
        for h in (0, 1):
            fillv = -1 if os.environ.get("GAT_NEGPAD") else 0
            s = np.full(max(LEN[h], 16), fillv, dtype=np.int16)
            m = sel & (e_half == h)
            lin = blk_off[e_k[m], h] + rank[m] * 128 + e_p[m]
            s[lin] = e_val[m]
            # equalize valid count per (k, h) block: flip pads to 0
            for k in range(KCH):
                need = int(K_valid[k, h] - nv[c, k, h])
                if need > 0:
                    b0 = blk_off[k, h]
                    blk = s[b0:b0 + int(D_uni[k, h]) * 128]
                    padpos = np.flatnonzero(blk < 0)[:need]
                    blk[padpos] = 0
            streams.append(_wrap16(s))
        idx_streams.append(streams)

        mk = np.zeros((128, CTOT), dtype=np.float32)
        mk[e_p[sel], col_off[e_k[sel], e_half[sel]] + rank[sel]] = 1.0
        masks.append(mk)

        own_rows = c * NPCP + np.arange(NPCP, dtype=np.int64)
        lo = c < NC // 2
        ov = (own_rows - (0 if lo else HALF)).astype(np.int16)
        zero = np.zeros(NPCP, dtype=np.int16)
        own_idx.append((_wrap16(ov if lo else zero), _wrap16(zero if lo else ov)))
        is_lo.append(np.full((128, 1), 1.0 if lo else 0.0, dtype=np.float32))

        bp = np.full((128, KCH), -1.0, dtype=np.float32)
        ic = np.zeros((128, KCH), dtype=np.float32)
        own_nodes = np.arange(c * NPC, (c + 1) * NPC)
        ppos = pos[own_nodes]
        bp[ppos % 128, ppos // 128] = batch[own_nodes].astype(np.float32)
        ic[ppos % 128, ppos // 128] = (1.0 / gcnt_c[batch[own_nodes]]).astype(np.float32)
        batchp.append(bp)
        invcnt.append(ic)

    xT = np.zeros((IN_CH, TROWS), dtype=np.float32)
    xT[:, rowid] = x.T

    W1aug = np.zeros((IN_CH, 260), dtype=np.float32)
    W1aug[:, :256] = W1
    W1aug[:, 256] = W1[:, 0:128] @ att_src1[0]
    W1aug[:, 257] = W1[:, 128:256] @ att_src1[1]
    W1aug[:, 258] = W1[:, 0:128] @ att_dst1[0]
    W1aug[:, 259] = W1[:, 128:256] @ att_dst1[1]
    W2aug = np.zeros((256, 130), dtype=np.float32)
    W2aug[:, :128] = W2
    W2aug[:, 128] = W2 @ att_src2[0]
    W2aug[:, 129] = W2 @ att_dst2[0]

    iota_row = np.tile(np.arange(G, dtype=np.float32), (128, 1))
    ident = np.eye(128, dtype=np.float32)

    n_valid = {h: {} for h in (0, 1)}   # per super: total valid count
    for h in (0, 1):
        for si, kl in enumerate(supers[h]):
            if os.environ.get("GAT_NEGPAD"):
                n_valid[h][si] = int(sum(K_valid[k, h] for k in kl))
            else:
                n_valid[h][si] = int(sum(D_uni[k, h] for k in kl)) * 128

    return dict(
        D_uni=D_uni, blk_off=blk_off, col_off=col_off, LEN=LEN, CTOT=CTOT,
        supers=supers, n_valid=n_valid, idx_streams=idx_streams, masks=masks,
        own_idx=own_idx, is_lo=is_lo, batchp=batchp, invcnt=invcnt, xT=xT,
        W1aug=W1aug, W2aug=W2aug, iota_row=iota_row, ident=ident,
        rowid=rowid, pos=pos,
    )


# --------------------------------------------------------------------------
# Numpy mirror of the device program (validation)
# --------------------------------------------------------------------------

def _np_aggregate(pp, table, elem, ncols, nheads, S, mask_c, idx_c, soff):
    D_uni, col_off = pp["D_uni"], pp["col_off"]
    OUT = np.zeros((128, KCH, nheads * ncols), dtype=np.float32)
    for k in range(KCH):
        acc = [np.zeros((128, ncols), dtype=np.float32) for _ in range(nheads)]
        den = [np.zeros((128, 1), dtype=np.float32) for _ in range(nheads)]
        for h in (0, 1):
            D = int(D_uni[k, h])
            if D == 0:
                continue
            lin = pp["blk_off"][k, h] + np.arange(D * 128)
            idxs = idx_c[h][lin % 16, lin // 16].astype(np.int64)
            F = table[np.maximum(idxs, 0) + h * HALF].reshape(
                D, 128, elem).transpose(1, 0, 2)
            mk = mask_c[:, col_off[k, h]:col_off[k, h] + D]
            for hd in range(nheads):
                asrc = F[:, :, nheads * ncols + hd]
                adst = S[:, k, soff + nheads + hd:soff + nheads + hd + 1]
                e = asrc + adst
                e = np.where(e > 0, e, NEG_SLOPE * e).astype(np.float32)
                xm = (np.exp(e) * mk).astype(np.float32)
                den[hd] += xm.sum(axis=1, keepdims=True)
                acc[hd] += np.einsum("pr,prc->pc", xm,
                                     F[:, :, hd * ncols:(hd + 1) * ncols],
                                     ).astype(np.float32)
        for hd in range(nheads):
            rc = (1.0 / (den[hd] + EPS)).astype(np.float32)
            OUT[:, k, hd * ncols:(hd + 1) * ncols] = acc[hd] * rc
    return OUT


def _bf(a):
    import ml_dtypes
    return a.astype(ml_dtypes.bfloat16).astype(np.float32)


def _numpy_forward(pp, b1, b2, lw1, lb1, lw2, lb2):
    table1 = np.zeros((TROWS, ELEM1), dtype=np.float32)
    table1[:, :260] = _bf(_bf(pp["xT"]).T @ _bf(pp["W1aug"]))

    t2own_all = []
    out1_all = {}
    for c in range(NC):
        ownrows = c * NPCP + np.arange(NPCP)
        # scores live at table cols 256:260 -> elem-cols 0:4 of the 128-col
        # score gather (cols 256:384)
        S1 = table1[ownrows][:, 256:384].reshape(KCH, 128, 128).transpose(1, 0, 2)
        idx_c = pp["idx_streams"][c]
        OUT1 = _np_aggregate(pp, table1, ELEM1, 128, 2, S1, pp["masks"][c],
                             idx_c, 0)
        OUT1 = np.maximum(OUT1 + b1[None, None, :], 0.0).astype(np.float32)
        o1 = OUT1.transpose(1, 0, 2).reshape(NPCP, 256)
        t2own = np.zeros((NPCP, ELEM2), dtype=np.float32)
        t2own[:, :130] = _bf(_bf(o1) @ _bf(pp["W2aug"]))
        t2own_all.append(t2own)
        out1_all[c] = OUT1

    table2 = np.concatenate(t2own_all, axis=0)

    pooledT = np.zeros((128, G), dtype=np.float32)
    for c in range(NC):
        ownrows = c * NPCP + np.arange(NPCP)
        S2 = table2[ownrows][:, 128:256].reshape(KCH, 128, 128).transpose(1, 0, 2)
        idx_c = pp["idx_streams"][c]
        OUT2 = _np_aggregate(pp, table2, ELEM2, 128, 1, S2, pp["masks"][c],
                             idx_c, 0)
        OUT2 = np.maximum(OUT2 + b2[None, None, :], 0.0).astype(np.float32)
        for k in range(KCH):
            o2s = OUT2[:, k, :] * pp["invcnt"][c][:, k:k + 1]
            onehot = (pp["iota_row"] == pp["batchp"][c][:, k:k + 1]).astype(np.float32)
            pooledT += o2s.T @ onehot

    z1 = np.maximum(lw1.T @ pooledT + lb1[:, None], 0.0)
    out = lw2.T @ z1 + lb2[:, None]
    return out.T.astype(np.float32)


# --------------------------------------------------------------------------
# Device program
# --------------------------------------------------------------------------

def _build_program(pp, lb2f):
    sys.path.insert(0, "/opt/trn_rl_repo")
    import concourse.bass as bass
    import concourse.tile as tile
    from concourse import bacc, mybir

    f32 = mybir.dt.float32
    bf16 = mybir.dt.bfloat16
    i16 = mybir.dt.int16
    AF = mybir.ActivationFunctionType
    ALU = mybir.AluOpType
    X = mybir.AxisListType.X
    D_uni = pp["D_uni"]
    col_off = pp["col_off"]
    supers = pp["supers"]
    n_valid = pp["n_valid"]
    LEN = pp["LEN"]
    CTOT = pp["CTOT"]

    nc = bacc.Bacc("TRN2", target_bir_lowering=False, debug=False, num_devices=NC)

    xT_d = nc.dram_tensor("xT", [IN_CH, TROWS], bf16, kind="ExternalInput")
    W1a_d = nc.dram_tensor("W1aug", [IN_CH, 260], bf16, kind="ExternalInput")
    W2a_d = nc.dram_tensor("W2aug", [256, 130], bf16, kind="ExternalInput")
    idx_d = {0: nc.dram_tensor("idx_lo", [128, max(LEN[0], 16) // 16], i16, kind="ExternalInput"),
             1: nc.dram_tensor("idx_hi", [128, max(LEN[1], 16) // 16], i16, kind="ExternalInput")}
    oidx_lo_d = nc.dram_tensor("oidx_lo", [128, NPCP // 16], i16, kind="ExternalInput")
    oidx_hi_d = nc.dram_tensor("oidx_hi", [128, NPCP // 16], i16, kind="ExternalInput")
    mask_d = nc.dram_tensor("mask", [128, CTOT], f32, kind="ExternalInput")
    islo_d = nc.dram_tensor("islo", [128, 1], f32, kind="ExternalInput")
    batchp_d = nc.dram_tensor("batchp", [128, KCH], f32, kind="ExternalInput")
    invcnt_d = nc.dram_tensor("invcnt", [128, KCH], f32, kind="ExternalInput")
    iota_d = nc.dram_tensor("iota", [128, G], f32, kind="ExternalInput")
    ident_d = nc.dram_tensor("ident", [128, 128], f32, kind="ExternalInput")
    b1_d = nc.dram_tensor("b1r", [128, 256], f32, kind="ExternalInput")
    b2_d = nc.dram_tensor("b2r", [128, 128], f32, kind="ExternalInput")
    lw1_d = nc.dram_tensor("lw1", [128, 64], f32, kind="ExternalInput")
    lb1_d = nc.dram_tensor("lb1", [64, 1], f32, kind="ExternalInput")
    lw2_d = nc.dram_tensor("lw2", [64, 1], f32, kind="ExternalInput")
    out_d = nc.dram_tensor("out", [1, G], f32, kind="ExternalOutput")

    with tile.TileContext(nc) as tc, ExitStack() as ctx:
        dr = ctx.enter_context(tc.tile_pool(name="dr", bufs=1, space="DRAM"))
        table1 = dr.tile([TROWS, ELEM1], bf16)
        table2own = dr.tile([NPCP, ELEM2], bf16)
        table2 = dr.tile([TROWS, ELEM2], bf16, addr_space="Shared")
        out1_dram = dr.tile([NPCP, 256], f32)
        out2_dram = dr.tile([NPCP, 128], f32)
        arin = dr.tile([128, G], f32)
        arout = dr.tile([128, G], f32)

        consts = ctx.enter_context(tc.tile_pool(name="consts", bufs=1))
        xchunk_p = ctx.enter_context(tc.tile_pool(name="xchunk", bufs=4))
        hps_p = ctx.enter_context(tc.tile_pool(name="hps", bufs=2, space="PSUM"))
        hrow_p = ctx.enter_context(tc.tile_pool(name="hrow", bufs=4))
        sg_p = ctx.enter_context(tc.tile_pool(name="sg", bufs=1))
        ssel_p = ctx.enter_context(tc.tile_pool(name="ssel", bufs=1))
        flo_p = ctx.enter_context(tc.tile_pool(name="flo", bufs=2))
        fhi_p = ctx.enter_context(tc.tile_pool(name="fhi", bufs=2))
        ilo_p = ctx.enter_context(tc.tile_pool(name="ilo", bufs=2))
        ihi_p = ctx.enter_context(tc.tile_pool(name="ihi", bufs=2))
        small_p = ctx.enter_context(tc.tile_pool(name="small", bufs=10))
        pk_p = ctx.enter_context(tc.tile_pool(name="pk", bufs=4))
        red_p = ctx.enter_context(tc.tile_pool(name="red", bufs=6))
        og_p = ctx.enter_context(tc.tile_pool(name="og", bufs=3))
        tps_p = ctx.enter_context(tc.tile_pool(name="tps", bufs=2, space="PSUM"))
        t2s_p = ctx.enter_context(tc.tile_pool(name="t2s", bufs=3))
        pool_ps = ctx.enter_context(tc.tile_pool(name="poolps", bufs=1, space="PSUM"))
        oh_p = ctx.enter_context(tc.tile_pool(name="oh", bufs=2))
        mlp_p = ctx.enter_context(tc.tile_pool(name="mlp", bufs=1))
        mlp_ps = ctx.enter_context(tc.tile_pool(name="mlpps", bufs=1, space="PSUM"))

        W1a_t = consts.tile([128, 260], bf16)
        nc.sync.dma_start(W1a_t[:], W1a_d[:, :])
        W2a_t = consts.tile([128, 2 * 130], bf16)
        nc.sync.dma_start(W2a_t[:, 0:130], W2a_d[0:128, :])
        nc.sync.dma_start(W2a_t[:, 130:260], W2a_d[128:256, :])
        mask_t = consts.tile([128, CTOT], f32)
        nc.sync.dma_start(mask_t[:], mask_d[:, :])
        islo_t = consts.tile([128, 1], f32)
        nc.sync.dma_start(islo_t[:], islo_d[:, :])
        batchp_t = consts.tile([128, KCH], f32)
        nc.sync.dma_start(batchp_t[:], batchp_d[:, :])
        invcnt_t = consts.tile([128, KCH], f32)
        nc.sync.dma_start(invcnt_t[:], invcnt_d[:, :])
        iota_t = consts.tile([128, G], f32)
        nc.sync.dma_start(iota_t[:], iota_d[:, :])
        ident_t = consts.tile([128, 128], f32)
        nc.sync.dma_start(ident_t[:], ident_d[:, :])
        b1_t = consts.tile([128, 256], f32)
        nc.sync.dma_start(b1_t[:], b1_d[:, :])
        b2_t = consts.tile([128, 128], f32)
        nc.sync.dma_start(b2_t[:], b2_d[:, :])
        lw1_t = consts.tile([128, 64], f32)
        nc.sync.dma_start(lw1_t[:], lw1_d[:, :])
        lb1_t = consts.tile([64, 1], f32)
        nc.sync.dma_start(lb1_t[:], lb1_d[:, :])
        lw2_t = consts.tile([64, 1], f32)
        nc.sync.dma_start(lw2_t[:], lw2_d[:, :])
        oilo_t = consts.tile([128, NPCP // 16], i16)
        nc.sync.dma_start(oilo_t[:], oidx_lo_d[:, :])
        oihi_t = consts.tile([128, NPCP // 16], i16)
        nc.sync.dma_start(oihi_t[:], oidx_hi_d[:, :])

        # ---- Phase A: replicated table1 ----
        NKK = TROWS // 128
        XB = 8
        for kb in range(NKK // XB):
            xc = xchunk_p.tile([128, XB * 128], bf16)
            nc.sync.dma_start(xc[:], xT_d[:, kb * XB * 128:(kb + 1) * XB * 128])
            for j in range(XB):
                kk = kb * XB + j
                ps = hps_p.tile([128, 260], f32)
                nc.tensor.matmul(ps[:], xc[:, j * 128:(j + 1) * 128], W1a_t[:],
                                 start=True, stop=True)
                hr = hrow_p.tile([128, ELEM1], bf16)
                if kk % 2 == 0:
                    nc.scalar.copy(hr[:, 0:260], ps[:])
                else:
                    nc.vector.tensor_copy(hr[:, 0:260], ps[:])
                nc.sync.dma_start(table1[kk * 128:(kk + 1) * 128, :], hr[:])

        # ---- score gathers (own rows; two halves + select) ----
        def score_gather(tab, elem, nheads, tag):
            tiles = {}
            for (oi, h) in ((oilo_t, 0), (oihi_t, 1)):
                sgt = sg_p.tile([128, KCH * 128], bf16, tag="sg")
                nc.gpsimd.dma_gather(
                    out_ap=sgt[:].rearrange("p (r e) -> p r e", e=128),
                    in_ap=tab[h * HALF:(h + 1) * HALF,
                              nheads * 128:nheads * 128 + 128],
                    idxs_ap=oi[:],
                    num_idxs=NPCP,
                    num_idxs_reg=NPCP,
                    elem_size=128,
                    elem_step=elem,
                    single_packet=False,
                )
                soff = 0
                cmp_t = ssel_p.tile([128, KCH * 4], f32, tag=f"cmp{tag}{h}")
                nc.vector.tensor_copy(
                    cmp_t[:].rearrange("p (r e) -> p r e", e=4),
                    sgt[:].rearrange("p (r e) -> p r e", e=128)[:, :, soff:soff + 4])
                tiles[h] = cmp_t
            S = ssel_p.tile([128, KCH * 4], f32, tag=f"S{tag}")
            nc.vector.tensor_tensor(S[:], tiles[0][:], tiles[1][:], ALU.subtract)
            nc.vector.tensor_scalar(S[:], S[:], islo_t[:, 0:1], None, ALU.mult)
            nc.vector.tensor_tensor(S[:], S[:], tiles[1][:], ALU.add)
            return S

        S1 = score_gather(table1, ELEM1, 2, "a")

        # ---- aggregation ----
        def aggregate(tab, elem, ncols, nheads, S, out_dram_t, bias_t, tag):
            sup_of_k = {}
            for h in (0, 1):
                for si, kl in enumerate(supers[h]):
                    off = 0
                    for k in kl:
                        sup_of_k[(k, h)] = (si, off)
                        off += int(D_uni[k, h])
            R_CAP = max(max(int(sum(D_uni[k, h] for k in kl)) for kl in supers[h])
                        for h in (0, 1))
            f_pools = {0: flo_p, 1: fhi_p}
            i_pools = {0: ilo_p, 1: ihi_p}
            cur_super = {0: -1, 1: -1}
            f_tiles = {}
            first_uses = {0: 0, 1: 0}

            def ensure_super(h, si):
                if cur_super[h] == si:
                    return
                kl = supers[h][si]
                rtot = int(sum(D_uni[k, h] for k in kl))
                start = int(pp["blk_off"][kl[0], h])
                nidx = rtot * 128
                it = i_pools[h].tile([128, nidx // 16], i16, tag=f"i{h}")
                nc.sync.dma_start(it[:], idx_d[h][:, start // 16:(start + nidx) // 16])
                ft = f_pools[h].tile([128, R_CAP * ELEM1], bf16, tag=f"f{h}")
                if first_uses[h] < 2:
                    nc.vector.memset(ft[:], 0.0)
                    first_uses[h] += 1
                nc.gpsimd.dma_gather(
                    out_ap=ft[:, 0:rtot * elem].rearrange("p (r e) -> p r e", e=elem),
                    in_ap=tab[h * HALF:(h + 1) * HALF, 0:elem],
                    idxs_ap=it[:],
                    num_idxs=nidx,
                    num_idxs_reg=n_valid[h][si],
                    elem_size=elem,
                    single_packet=False,
                )
                f_tiles[h] = ft
                cur_super[h] = si

            lite = os.environ.get("GAT_LITE", "")
            for k in range(KCH):
                dens = {}
                reds = {}
                for h in (0, 1):
                    D = int(D_uni[k, h])
                    if D == 0:
                        continue
                    si, roff = sup_of_k[(k, h)]
                    ensure_super(h, si)
                    if lite == "gath":
                        continue
                    F3 = f_tiles[h][:, roff * elem:(roff + D) * elem].rearrange(
                        "p (r e) -> p r e", e=elem)
                    for hd in range(nheads):
                        e_t = small_p.tile([128, D], f32, tag="e")
                        nc.scalar.activation(
                            e_t[:], F3[:, :, nheads * ncols + hd],
                            AF.Prelu,
                            bias=S[:, k * 4 + nheads + hd:k * 4 + nheads + hd + 1],
                            scale=1.0, alpha=NEG_SLOPE)
                        x_t = small_p.tile([128, D], f32, tag="x")
                        nc.scalar.activation(x_t[:], e_t[:], AF.Exp)
                        xm = small_p.tile([128, D], f32, tag="xm")
                        nc.vector.tensor_tensor(
                            xm[:], x_t[:],
                            mask_t[:, col_off[k, h]:col_off[k, h] + D], ALU.mult)
                        d_t = small_p.tile([128, 1], f32, tag="d")
                        nc.vector.tensor_reduce(d_t[:], xm[:], X, ALU.add)
                        dens[(h, hd)] = d_t
                        if lite == "eops":
                            continue
                        # transposed-packed products: pk[p, c*D + r]
                        pk = pk_p.tile([128, R_CAP * ncols], f32, tag="pk")
                        pkv = pk[:, 0:D * ncols].rearrange("p (c r) -> p c r", r=D)
                        for r in range(D):
                            if r % 2 == 1:
                                nc.vector.tensor_scalar(
                                    pkv[:, :, r],
                                    F3[:, r, hd * ncols:(hd + 1) * ncols],
                                    xm[:, r:r + 1], None, ALU.mult)
                            else:
                                nc.scalar.activation(
                                    pkv[:, :, r],
                                    F3[:, r, hd * ncols:(hd + 1) * ncols],
                                    AF.Copy, scale=xm[:, r:r + 1])
                        red = red_p.tile([128, ncols], f32, tag="red")
                        nc.vector.tensor_reduce(red[:], pkv, X, ALU.add)
                        reds[(h, hd)] = red
                og = og_p.tile([128, nheads * ncols], f32, tag="og")
                if lite in ("gath", "eops"):
                    nc.vector.memset(og[:], 0.0)
                    nc.sync.dma_start(out_dram_t[k * 128:(k + 1) * 128, :], og[:])
                    continue
                for hd in range(nheads):
                    have = [h for h in (0, 1) if (h, hd) in reds]
                    r0 = reds[(have[0], hd)]
                    d0 = dens[(have[0], hd)]
                    if len(have) == 2:
                        nc.vector.tensor_tensor(r0[:], r0[:], reds[(have[1], hd)][:],
                                                ALU.add)
                        nc.vector.tensor_tensor(d0[:], d0[:], dens[(have[1], hd)][:],
                                                ALU.add)
                    nc.vector.tensor_scalar(d0[:], d0[:], EPS, None, ALU.add)
                    rc = small_p.tile([128, 1], f32, tag="rc")
                    nc.vector.reciprocal(rc[:], d0[:])
                    nc.scalar.activation(og[:, hd * ncols:(hd + 1) * ncols], r0[:],
                                         AF.Copy, scale=rc[:, 0:1])
                nc.vector.tensor_tensor(og[:], og[:], bias_t[:, 0:nheads * ncols],
                                        ALU.add)
                nc.scalar.activation(og[:], og[:], AF.Relu)
                nc.sync.dma_start(
                    out_dram_t[k * 128:(k + 1) * 128, :], og[:])

        stage = os.environ.get("GAT_STAGE", "full")
        slvl = {"A": 0, "C": 1, "D": 2, "E": 3, "F": 4, "full": 9}[stage]

        if slvl >= 1:
            aggregate(table1, ELEM1, 128, 2, S1, out1_dram, b1_t, "a")

        if slvl >= 2:
            # ---- Phase D: layer-2 table ----
            for k in range(KCH):
                o1g = t2s_p.tile([128, 256], f32, tag="o1g")
                nc.sync.dma_start(o1g[:], out1_dram[k * 128:(k + 1) * 128, :])
                o1T = {}
                for half in (0, 1):
                    tp = tps_p.tile([128, 128], f32, tag="tp")
                    nc.tensor.transpose(
                        tp[:], o1g[:, half * 128:(half + 1) * 128], ident_t[:])
                    st = t2s_p.tile([128, 128], bf16, tag=f"o1T{half}")
                    nc.scalar.copy(st[:], tp[:])
                    o1T[half] = st
                ps2 = tps_p.tile([128, 130], f32, tag="tp")
                nc.tensor.matmul(ps2[:], o1T[0][:], W2a_t[:, 0:130],
                                 start=True, stop=False)
                nc.tensor.matmul(ps2[:], o1T[1][:], W2a_t[:, 130:260],
                                 start=False, stop=True)
                h2r = t2s_p.tile([128, ELEM2], bf16, tag="h2r")
                nc.vector.tensor_copy(h2r[:, 0:130], ps2[:])
                nc.sync.dma_start(table2own[k * 128:(k + 1) * 128, :], h2r[:])

        if slvl >= 3:
            # ---- Phase E ----
            nc.gpsimd.collective_compute(
                "AllGather", mybir.AluOpType.bypass,
                replica_groups=[list(range(NC))],
                ins=[table2own[:].opt()],
                outs=[table2[:].opt()],
            )
            S2 = score_gather(table2, ELEM2, 1, "b")

        if slvl >= 4:
            # ---- Phase F ----
            aggregate(table2, ELEM2, 128, 1, S2, out2_dram, b2_t, "b")

        if slvl < 9:
            orow0 = mlp_p.tile([1, G], f32, tag="orow")
            nc.vector.memset(orow0[:], 0.0)
            nc.sync.dma_start(out_d[:, :], orow0[:])
        else:
            # ---- Phase G: pooling ----
            psA = pool_ps.tile([128, 512], f32, tag="psA")
            psB = pool_ps.tile([128, 512], f32, tag="psB")
            for k in range(KCH):
                o2g = oh_p.tile([128, 128], f32, tag="o2g")
                nc.sync.dma_start(o2g[:], out2_dram[k * 128:(k + 1) * 128, :])
                o2s = oh_p.tile([128, 128], f32, tag="o2s")
                nc.scalar.activation(o2s[:], o2g[:], AF.Copy,
                                     scale=invcnt_t[:, k:k + 1])
                onehot = oh_p.tile([128, G], f32, tag="onehot")
                nc.vector.tensor_scalar(onehot[:], iota_t[:], batchp_t[:, k:k + 1],
                                        None, ALU.is_equal)
                nc.tensor.matmul(psA[:], o2s[:], onehot[:, 0:512],
                                 start=(k == 0), stop=(k == KCH - 1))
                nc.tensor.matmul(psB[:], o2s[:], onehot[:, 512:1024],
                                 start=(k == 0), stop=(k == KCH - 1))
            pooledT = mlp_p.tile([128, G], f32, tag="pooledT")
            nc.vector.tensor_copy(pooledT[:, 0:512], psA[:])
            nc.vector.tensor_copy(pooledT[:, 512:1024], psB[:])
            nc.sync.dma_start(arin[:], pooledT[:])
            nc.gpsimd.collective_compute(
                "AllReduce", mybir.AluOpType.add,
                replica_groups=[list(range(NC))],
                ins=[arin[:].opt()],
                outs=[arout[:].opt()],
            )
            pooled2 = mlp_p.tile([128, G], f32, tag="pooled2")
            nc.sync.dma_start(pooled2[:], arout[:])

            # ---- Phase I: MLP ----
            z1 = mlp_p.tile([64, G], f32, tag="z1")
            for half in (0, 1):
                zps = mlp_ps.tile([64, 512], f32, tag="m")
                nc.tensor.matmul(zps[:], lw1_t[:],
                                 pooled2[:, half * 512:(half + 1) * 512],
                                 start=True, stop=True)
                nc.scalar.activation(z1[:, half * 512:(half + 1) * 512], zps[:],
                                     AF.Relu, bias=lb1_t[:, 0:1], scale=1.0)
            orow = mlp_p.tile([1, G], f32, tag="orow")
            for half in (0, 1):
                ops_full = mlp_ps.tile([64, 512], f32, tag="m")
                ops_ = ops_full[0:1, :]
                nc.tensor.matmul(ops_, lw2_t[:], z1[:, half * 512:(half + 1) * 512],
                                 start=True, stop=True)
                nc.scalar.activation(orow[:, half * 512:(half + 1) * 512], ops_,
                                     AF.Copy, bias=lb2f, scale=1.0)
            nc.sync.dma_start(out_d[:, :], orow[:])

    nc.compile()
    return nc


# --------------------------------------------------------------------------
# Entry point
# --------------------------------------------------------------------------

def kernel(x, edge_index, batch, num_graphs, W1, att_src1, att_dst1, b1,
           W2, att_src2, att_dst2, b2, lw1, lb1, lw2, lb2):
    import ml_dtypes
    bfnp = ml_dtypes.bfloat16

    x = np.asarray(x, dtype=np.float32)
    edge_index = np.asarray(edge_index, dtype=np.int64)
    batch = np.asarray(batch, dtype=np.int64)
    W1 = np.asarray(W1, dtype=np.float32)
    att_src1 = np.asarray(att_src1, dtype=np.float32)
    att_dst1 = np.asarray(att_dst1, dtype=np.float32)
    b1 = np.asarray(b1, dtype=np.float32)
    W2 = np.asarray(W2, dtype=np.float32)
    att_src2 = np.asarray(att_src2, dtype=np.float32)
    att_dst2 = np.asarray(att_dst2, dtype=np.float32)
    b2 = np.asarray(b2, dtype=np.float32)
    lw1 = np.asarray(lw1, dtype=np.float32)
    lb1 = np.asarray(lb1, dtype=np.float32)
    lw2 = np.asarray(lw2, dtype=np.float32)
    lb2 = np.asarray(lb2, dtype=np.float32)
    assert x.shape == (N, IN_CH) and edge_index.shape == (2, E)
    assert int(num_graphs) == G

    _log("prep...")
    pp = _prep(x, edge_index, batch, W1, att_src1, att_dst1, W2, att_src2,
               att_dst2)

    if os.environ.get("GAT_NUMPY_ONLY"):
        return _numpy_forward(pp, b1, b2, lw1, lb1, lw2, lb2)

    _log("build+compile...")
    nc = _build_program(pp, float(lb2[0]))

    from concourse.bass_utils import run_bass_kernel_spmd

    in_maps = []
    for c in range(NC):
        in_maps.append({
            "xT": pp["xT"].astype(bfnp), "W1aug": pp["W1aug"].astype(bfnp),
            "W2aug": pp["W2aug"].astype(bfnp),
            "idx_lo": pp["idx_streams"][c][0], "idx_hi": pp["idx_streams"][c][1],
            "oidx_lo": pp["own_idx"][c][0], "oidx_hi": pp["own_idx"][c][1],
            "mask": pp["masks"][c], "islo": pp["is_lo"][c],
            "batchp": pp["batchp"][c], "invcnt": pp["invcnt"][c],
            "iota": pp["iota_row"], "ident": pp["ident"],
            "b1r": np.tile(b1, (128, 1)), "b2r": np.tile(b2, (128, 1)),
            "lw1": lw1, "lb1": lb1.reshape(64, 1), "lw2": lw2,
        })
    _log("run...")
    import time as _time
    res = run_bass_kernel_spmd(nc, in_maps, list(range(NC)))
    global LAST_EXEC_TIME_NS
    if os.environ.get("GAT_TIME"):
        best = None
        for _ in range(2):
            t0 = _time.perf_counter()
            res = run_bass_kernel_spmd(nc, in_maps, list(range(NC)))
            dt = _time.perf_counter() - t0
            best = dt if best is None else min(best, dt)
        LAST_EXEC_TIME_NS = int(best * 1e9)
        _log("repeat-run wall (upper bound on HW):", best)
    out = res.results[0]["out"]
    return out.reshape(G, 1).astype(np.float32)

